# revision 1
# baseline (speedup 1.0000x reference)
"""AttentionHead kernel for 8 Trainium2 NeuronCores (SPMD data-parallel).

Problem: q/k/v projections [1024->64] + masked softmax attention,
B=4, S=2048, d_model=1024, d_k=64.

Sharding: 8 cores = 4 batches x 2 query-halves. Each core handles one
(batch, q-half): query shard [1024, 1024], full key/value for its batch
[2048, 1024], mask shard [1024, 2048]. Weights replicated.

Per-core device pipeline (everything contracts on the partition dim;
all inputs host-packed so each DMA is one large contiguous transfer):
  - projections: qT [64, sq], kT [64, skv] via matmul(lhsT=w_t, rhs=xT)
  - v projected per skv-block to natural [128, 64] (lhsT=valueT block),
    augmented with a ones column
  - scores computed TRANSPOSED [skv_tile=128, sq=1024]: no probability
    transpose needed anywhere
  - ACT exp reads scores straight from PSUM with the 1024**-0.5 scale
    fused; masked positions then set to 1.0 (==exp(1e-9) in fp32) via
    copy_predicated on the bf16 E tile
  - PV accumulates transposed: oT[65, sq] += vaug_j.T @ E_j; row 64 is
    the softmax denominator (free via the ones column)
  - finalize: PE-transpose oT back to [sq, 65], multiply by reciprocal
    of column 64, one batched output DMA
"""

import numpy as np
import ml_dtypes

B = 4
S = 2048
D_MODEL = 1024
D_K = 64
N_CORES = 8

P = 128
SQ = S // 2          # per-core query rows (1024)
SKV = S              # per-core kv rows (2048)
MB = D_MODEL // P    # 8 m-blocks (contraction)
JT = SKV // P        # 16 skv tiles
IT = SQ // P         # 8 sq tiles
NQC = SQ // 512      # 2 q chunks
NKC = SKV // 512     # 4 k chunks
VG = 4               # v/mask DMA groups
JPG = JT // VG       # skv tiles per DMA group

_BF16 = ml_dtypes.bfloat16

_cached_nc = None


def _build_nc():
    import concourse.mybir as mybir
    import concourse.tile as tile
    from concourse import bacc

    bf16 = mybir.dt.bfloat16
    f32 = mybir.dt.float32
    u8 = mybir.dt.uint8

    nc = bacc.Bacc(None, target_bir_lowering=False)

    w_d = nc.dram_tensor("w_all", [P, MB, 3 * D_K], bf16, kind="ExternalInput")
    q_d = nc.dram_tensor("q_t", [P, MB, SQ], bf16, kind="ExternalInput")
    k_d = nc.dram_tensor("k_t", [P, MB, SKV], bf16, kind="ExternalInput")
    v_d = nc.dram_tensor("v_t", [P, JT, MB, P], bf16, kind="ExternalInput")
    m_d = nc.dram_tensor("mask_t", [P, JT, SQ], u8, kind="ExternalInput")
    idn_d = nc.dram_tensor("idn_t", [D_K + 1, D_K + 1], f32, kind="ExternalInput")
    out_d = nc.dram_tensor("out", [P, IT, D_K], f32, kind="ExternalOutput")

    with tile.TileContext(nc) as tc:
        with (
            tc.tile_pool(name="const", bufs=1) as cpool,
            tc.tile_pool(name="inp", bufs=1) as ipool,
            tc.tile_pool(name="proj", bufs=1) as jpool,
            tc.tile_pool(name="fin", bufs=2) as fpool,
            tc.tile_pool(name="ps_pqk", bufs=1, space="PSUM") as ps_pqk,
            tc.tile_pool(name="ps_pv", bufs=2, space="PSUM") as ps_pv,
            tc.tile_pool(name="ps_s", bufs=3, space="PSUM") as ps_s,
            tc.tile_pool(name="ps_o", bufs=1, space="PSUM") as ps_o,
        ):
            # ---- input DMAs: few, large, streamed in compute order ----
            w_sb = cpool.tile([P, MB, 3 * D_K], bf16, tag="w")
            nc.sync.dma_start(out=w_sb, in_=w_d[:])
            q_sb = ipool.tile([P, MB, SQ], bf16, tag="q")
            nc.sync.dma_start(out=q_sb, in_=q_d[:])

            kcs = [None] * NKC
            mgs = [None] * VG
            vgs = [None] * VG

            def dma_k(t):
                kc = ipool.tile([P, MB, 512], bf16, tag=f"k{t}", name=f"k{t}")
                nc.sync.dma_start(out=kc, in_=k_d[:, :, t * 512 : (t + 1) * 512])
                kcs[t] = kc

            def dma_m(g):
                mg = ipool.tile([P, JPG, SQ], u8, tag=f"m{g}", name=f"m{g}")
                nc.sync.dma_start(out=mg, in_=m_d[:, g * JPG : (g + 1) * JPG, :])
                mgs[g] = mg

            def dma_v(g):
                vg = ipool.tile([P, JPG, MB, P], bf16, tag=f"v{g}", name=f"v{g}")
                nc.sync.dma_start(out=vg, in_=v_d[:, g * JPG : (g + 1) * JPG, :, :])
                vgs[g] = vg

            # stream order tuned to dependency-ready times
            dma_k(0)
            dma_m(0)
            dma_k(1)
            dma_v(0)
            dma_k(2)
            dma_m(1)
            dma_v(1)
            dma_k(3)
            dma_m(2)
            dma_m(3)
            dma_v(2)
            dma_v(3)

            def wq(i):
                return w_sb[:, i, 0:D_K]

            def wk(i):
                return w_sb[:, i, D_K : 2 * D_K]

            def wv(i):
                return w_sb[:, i, 2 * D_K : 3 * D_K]

            # ---- constants ----
            ones_bf = cpool.tile([P, 2, SQ], bf16, tag="ones")
            nc.vector.memset(ones_bf, 1.0)
            idn = cpool.tile([D_K + 1, D_K + 1], f32, tag="idn")
            nc.sync.dma_start(out=idn, in_=idn_d[:])

            # ---- PE warm-up: keep the HAM clock at 2.4 GHz before real
            # matmuls start (burns idle PE time while DMAs stream) ----
            warm_ps = ps_pqk.tile([D_K, 512], f32, tag="pqk", name="warm")
            w_flat = w_sb.rearrange("p mb k -> p (mb k)")
            for wi in range(16):
                nc.tensor.matmul(
                    warm_ps,
                    lhsT=wq(0),
                    rhs=w_flat[:, 0:512],
                    start=(wi == 0),
                    stop=(wi == 15),
                )

            # ---- q/k projections ----
            qTd = jpool.tile([D_K, SQ], bf16, tag="qT")
            kTd = jpool.tile([D_K, SKV], bf16, tag="kT")

            def kproj(t):
                pp = ps_pqk.tile([D_K, 512], f32, tag="pqk", name=f"pk{t}")
                for i in range(MB):
                    nc.tensor.matmul(
                        pp,
                        lhsT=wk(i),
                        rhs=kcs[t][:, i, :],
                        start=(i == 0),
                        stop=(i == MB - 1),
                    )
                sl = slice(t * 512, (t + 1) * 512)
                nc.vector.tensor_copy(kTd[:, sl], pp)

            for t in range(NQC):
                pp = ps_pqk.tile([D_K, 512], f32, tag="pqk", name=f"pq{t}")
                for i in range(MB):
                    nc.tensor.matmul(
                        pp,
                        lhsT=wq(i),
                        rhs=q_sb[:, i, t * 512 : (t + 1) * 512],
                        start=(i == 0),
                        stop=(i == MB - 1),
                    )
                sl = slice(t * 512, (t + 1) * 512)
                nc.vector.tensor_copy(qTd[:, sl], pp)
            kproj(0)

            # ---- per-j v-aug tiles (col 64 = ones) ----
            vaugs = []
            for j in range(JT):
                va = jpool.tile([P, D_K + 1], bf16, tag=f"va{j}", name=f"va{j}")
                nc.vector.memset(va[:, D_K : D_K + 1], 1.0)
                vaugs.append(va)

            # ---- per-pair E tiles [128, 2, 1024] ----
            Eps = [
                jpool.tile([P, 2, SQ], bf16, tag=f"E{p}", name=f"E{p}")
                for p in range(JT // 2)
            ]

            def E_of(j2):
                return Eps[j2 // 2][:, j2 % 2, :]

            # ---- transposed output accumulator [65, 1024] f32 = 2 banks,
            # one accumulation group per bank ----
            oTp = ps_o.tile([D_K + 1, SQ], f32, tag="oT")

            # ---- main pipeline over skv tiles.
            # Emission order is engine-queue order. Cross-engine consumers
            # are emitted a few iterations late (vproj @ j-2, PV @ j-4) so
            # the in-order PE stream never stalls on the DVE/ACT chain.
            pvs = [None] * JT

            def emit_vproj(j2):
                g, jj = divmod(j2, JPG)
                pv = ps_pv.tile([P, D_K], f32, tag="pv", name=f"pv{j2}")
                for i in range(MB):
                    nc.tensor.matmul(
                        pv,
                        lhsT=vgs[g][:, jj, i, :],
                        rhs=wv(i),
                        start=(i == 0),
                        stop=(i == MB - 1),
                    )
                pvs[j2] = pv

            def emit_cast(j2):
                nc.vector.tensor_copy(vaugs[j2][:, 0:D_K], pvs[j2])

            def emit_pv(j2):
                for c in range(NQC):
                    nc.tensor.matmul(
                        oTp[:, c * 512 : (c + 1) * 512],
                        lhsT=vaugs[j2],
                        rhs=E_of(j2)[:, c * 512 : (c + 1) * 512],
                        start=(j2 == 0),
                        stop=(j2 == JT - 1),
                    )

            for j in range(JT):
                g, jj = divmod(j, JPG)

                # transposed scores [skv_tile 128, sq] as two concurrent
                # row-group matmuls into separate single-bank psum tiles
                spa = ps_s.tile([P, 512], f32, tag="sp", name=f"spa{j}")
                spb = ps_s.tile([P, 512], f32, tag="sp", name=f"spb{j}")
                jsl = slice(j * P, (j + 1) * P)
                nc.tensor.matmul(
                    spa, lhsT=kTd[:, jsl], rhs=qTd[:, 0:512], start=True, stop=True
                )
                nc.tensor.matmul(
                    spb, lhsT=kTd[:, jsl], rhs=qTd[:, 512:1024], start=True, stop=True
                )
                if jj == 0 and g + 1 < NKC:
                    kproj(g + 1)  # one group ahead, after this group's scores
                if j >= 2:
                    emit_vproj(j - 2)
                if j >= 4:
                    emit_pv(j - 4)

                # E = exp(s / sqrt(d_model)) straight from PSUM, cast bf16
                for half, sp in ((0, spa), (1, spb)):
                    nc.scalar.activation(
                        out=E_of(j)[:, half * 512 : (half + 1) * 512],
                        in_=sp,
                        func=mybir.ActivationFunctionType.Exp,
                        scale=float(D_MODEL) ** -0.5,
                    )
                if j >= 3:
                    emit_cast(j - 3)
                if j % 2 == 1:
                    # masked positions -> 1.0 (== exp(1e-9) in fp32);
                    # one DVE op per pair of skv tiles
                    nc.vector.copy_predicated(
                        out=Eps[j // 2],
                        mask=mgs[g][:, jj - 1 : jj + 1, :],
                        data=ones_bf,
                    )

            for j2 in (JT - 2, JT - 1):
                emit_vproj(j2)
            for j2 in (JT - 3, JT - 2, JT - 1):
                emit_cast(j2)
            for j2 in range(JT - 4, JT):
                emit_pv(j2)

            # ---- finalize: transpose oT back (2 blocks per psum bank),
            # batched reciprocals, divide by ones-row ----
            oT_sb = jpool.tile([D_K + 1, SQ], f32, tag="oTs")
            nc.vector.tensor_copy(oT_sb, oTp)
            ob = fpool.tile([P, IT, D_K], f32, tag="ob", bufs=1)
            tps = []
            for t in range(IT // 2):
                tp = ps_s.tile([P, 2, D_K + 1], f32, tag="sp", name=f"tp{t}")
                for h in range(2):
                    i = 2 * t + h
                    nc.tensor.transpose(
                        tp[:, h, :], in_=oT_sb[:, i * P : (i + 1) * P], identity=idn
                    )
                tps.append(tp)
            for t in range(IT // 2):
                r2 = fpool.tile([P, 2], f32, tag="r")
                nc.vector.reciprocal(r2, tps[t][:, :, D_K])
                for h in range(2):
                    i = 2 * t + h
                    nc.vector.tensor_scalar_mul(
                        ob[:, i, :], tps[t][:, h, 0:D_K], r2[:, h : h + 1]
                    )
            nc.sync.dma_start(out=out_d[:], in_=ob)

    nc.finalize()
    return nc


def _get_nc():
    global _cached_nc
    if _cached_nc is None:
        _cached_nc = _build_nc()
    return _cached_nc


def _pack_mb(x_t):
    """[D_MODEL, s] -> [128, MB, s] (m-block packed, contiguous)."""
    s = x_t.shape[1]
    return np.ascontiguousarray(x_t.reshape(MB, P, s).transpose(1, 0, 2))


def _shard_inputs(query, key, value, mask, w_q, w_k, w_v):
    """Host-side shard + layout prep. Core c -> (batch c//2, q-half c%2)."""
    w_all = np.concatenate(
        [
            w.T.astype(_BF16).reshape(MB, P, D_K).transpose(1, 0, 2)
            for w in (w_q, w_k, w_v)
        ],
        axis=2,
    )
    w_all = np.ascontiguousarray(w_all)
    in_maps = []
    for c in range(N_CORES):
        b, h = divmod(c, 2)
        s0 = h * SQ
        q_t = query[b, s0 : s0 + SQ, :].T.astype(_BF16)
        k_t = key[b].T.astype(_BF16)
        v_t = value[b].T.astype(_BF16)
        m_t = mask[b, s0 : s0 + SQ, :].T.astype(np.uint8)
        in_maps.append(
            {
                "w_all": w_all,
                "q_t": _pack_mb(q_t),
                "k_t": _pack_mb(k_t),
                # [m, skv] -> [128, JT, MB, 128]: [p][j][i][s']
                "v_t": np.ascontiguousarray(
                    v_t.reshape(MB, P, JT, P).transpose(1, 2, 0, 3)
                ),
                # [skv, sq] -> [128, JT, SQ]
                "mask_t": np.ascontiguousarray(
                    m_t.reshape(JT, P, SQ).transpose(1, 0, 2)
                ),
                "idn_t": np.eye(D_K + 1, dtype=np.float32),
            }
        )
    return in_maps


def run(inputs, trace=False):
    """Run the SPMD kernel; returns (output [B,S,D_K] f32, BassKernelResults)."""
    from concourse.bass_utils import run_bass_kernel_spmd

    nc = _get_nc()
    in_maps = _shard_inputs(**inputs)
    res = run_bass_kernel_spmd(
        nc, in_maps, core_ids=list(range(N_CORES)), trace=trace
    )
    out = np.empty((B, S, D_K), np.float32)
    for c in range(N_CORES):
        b, h = divmod(c, 2)
        # device out is [128, IT, 64]: row = i*128+p
        o = res.results[c]["out"].transpose(1, 0, 2).reshape(SQ, D_K)
        out[b, h * SQ : (h + 1) * SQ, :] = o
    return out, res


def kernel(**inputs):
    out, _ = run(inputs, trace=False)
    return out



# revision 2
# speedup vs baseline: 1.1171x; 1.1171x over previous
"""AttentionHead kernel v5 for 8 Trainium2 NeuronCores (SPMD data-parallel).

Sharding (kv-shard): 8 cores = 4 batches x 2 KV-halves. Each core:
full query [2048, 1024] in FP8(e4m3), half key/value [1024, 1024] bf16,
inverted mask u8 [skv, sq]. Per-core DMA ~8.3 MiB.

Per-core pipeline (chunk c = 512 query columns, group g = 4 j-tiles):
  - kT/qT projections (M=64); qproj runs in fp8 (query + w_q quantized
    e4m3; only the q side is fp8 -- error ~1.3% rel, within the 2e-2
    budget); PSUM->SBUF evictions on ACT.
  - vproj: all 8 j into ONE PSUM bank, single DVE cast into vaug
    [128, 8, 65] (ones col).
  - scores per (j, c): [128, 512] transposed, pair PSUM tiles
    [128, 2, 512]; mask fused into the PSUM->SBUF eviction: ONE DVE
    tensor_tensor per pair: sb = scores * m' (u8). Masked -> exp(0)=1.
  - ACT exp per (g, c) on [128, 4, 512] SBUF bf16 (FD=2048).
  - PV: oT_c[65, 512] += vaug_j^T @ E; row 64 = denominator; oT
    eviction on DVE.
  - host: numer = oT[0:64], denom = oT[64], combine halves, divide.
"""

import numpy as np
import ml_dtypes

B = 4
S = 2048
D_MODEL = 1024
D_K = 64
N_CORES = 8

P = 128
SQ = S
SKV = S // 2
MB = D_MODEL // P
JT = SKV // P
NQC = SQ // 512
NKC = SKV // 512
NG = 2  # j-tile groups per chunk (4 j each)

_BF16 = ml_dtypes.bfloat16
_FP8 = ml_dtypes.float8_e4m3

_cached_nc = None


def _build_nc():
    import concourse.mybir as mybir
    import concourse.tile as tile
    from concourse import bacc

    bf16 = mybir.dt.bfloat16
    fp8 = mybir.dt.float8e4
    f32 = mybir.dt.float32
    u8 = mybir.dt.uint8
    Alu = mybir.AluOpType

    nc = bacc.Bacc(None, target_bir_lowering=False)

    wq_d = nc.dram_tensor("w_q", [P, MB, D_K], fp8, kind="ExternalInput")
    wkv_d = nc.dram_tensor("w_kv", [P, MB, 2 * D_K], bf16, kind="ExternalInput")
    q_d = nc.dram_tensor("q_t", [P, NQC, MB, 512], fp8, kind="ExternalInput")
    k_d = nc.dram_tensor("k_t", [P, NKC, MB, 512], bf16, kind="ExternalInput")
    v_d = nc.dram_tensor("v_t", [P, JT, MB, P], bf16, kind="ExternalInput")
    m_d = nc.dram_tensor("m_t", [P, NQC, JT, 512], u8, kind="ExternalInput")
    out_d = nc.dram_tensor("out", [D_K + 1, NQC, 512], bf16, kind="ExternalOutput")

    with tile.TileContext(nc) as tc:
        with (
            tc.tile_pool(name="const", bufs=1) as cpool,
            tc.tile_pool(name="inp", bufs=1) as ipool,
            tc.tile_pool(name="proj", bufs=1) as jpool,
            tc.tile_pool(name="ee", bufs=2) as epool,
            tc.tile_pool(name="fin", bufs=1) as fpool,
            tc.tile_pool(name="ps_pqk", bufs=2, space="PSUM") as ps_pqk,
            tc.tile_pool(name="ps_s", bufs=2, space="PSUM") as ps_s,
            tc.tile_pool(name="ps_o", bufs=1, space="PSUM") as ps_o,
            tc.tile_pool(name="ps_v", bufs=1, space="PSUM") as ps_v,
        ):
            k_sb = ipool.tile([P, NKC, MB, 512], bf16, tag="k")
            q_sb = ipool.tile([P, NQC, MB, 512], fp8, tag="q")
            m_sb = ipool.tile([P, NQC, JT, 512], u8, tag="m")
            v_sb = ipool.tile([P, JT, MB, P], bf16, tag="v")
            wq_sb = cpool.tile([P, MB, D_K], fp8, tag="wq")
            wkv_sb = cpool.tile([P, MB, 2 * D_K], bf16, tag="wkv")

            def dma_k(kc):
                nc.sync.dma_start(out=k_sb[:, kc], in_=k_d[:, kc])

            def dma_q(c):
                nc.sync.dma_start(out=q_sb[:, c], in_=q_d[:, c])

            def dma_m(c):
                nc.sync.dma_start(out=m_sb[:, c], in_=m_d[:, c])

            def dma_v(g):
                nc.sync.dma_start(
                    out=v_sb[:, g * 4 : (g + 1) * 4], in_=v_d[:, g * 4 : (g + 1) * 4]
                )

            dma_k(0)
            nc.sync.dma_start(out=wkv_sb, in_=wkv_d[:])
            nc.sync.dma_start(out=wq_sb, in_=wq_d[:])
            dma_q(0)
            dma_m(0)
            dma_k(1)
            dma_q(1)
            dma_m(1)
            dma_q(2)
            dma_m(2)
            dma_v(0)
            dma_v(1)
            dma_q(3)
            dma_m(3)

            def wq(i):
                return wq_sb[:, i]

            def wk(i):
                return wkv_sb[:, i, 0:D_K]

            def wv(i):
                return wkv_sb[:, i, D_K : 2 * D_K]

            warm = cpool.tile([P, 512], bf16, tag="warm")
            nc.vector.memset(warm, 0.25)
            vaug = jpool.tile([P, JT, D_K + 1], bf16, tag="vaug")
            nc.vector.memset(vaug[:, :, D_K : D_K + 1], 1.0)

            warm_ps = ps_pqk.tile([D_K, 512], f32, tag="pqk", name="warm")
            for wi in range(6):
                nc.tensor.matmul(
                    warm_ps,
                    lhsT=warm[:, 0:D_K],
                    rhs=warm,
                    start=(wi == 0),
                    stop=(wi == 5),
                )

            qT = jpool.tile([D_K, SQ], bf16, tag="qT")
            kT = jpool.tile([D_K, SKV], bf16, tag="kT")

            def kproj(kc):
                pp = ps_pqk.tile([D_K, 512], f32, tag="pqk", name=f"pk{kc}")
                for i in range(MB):
                    nc.tensor.matmul(
                        pp, lhsT=wk(i), rhs=k_sb[:, kc, i], start=(i == 0), stop=(i == MB - 1)
                    )
                nc.scalar.copy(kT[:, kc * 512 : (kc + 1) * 512], pp)

            def qproj(c):
                pp = ps_pqk.tile([D_K, 512], f32, tag="pqk", name=f"pq{c}")
                for i in range(MB):
                    nc.tensor.matmul(
                        pp, lhsT=wq(i), rhs=q_sb[:, c, i], start=(i == 0), stop=(i == MB - 1)
                    )
                nc.scalar.copy(qT[:, c * 512 : (c + 1) * 512], pp)

            def vproj():
                pv = ps_v.tile([P, JT, D_K], f32, tag="pv", name="pv")
                for j in range(JT):
                    for i in range(MB):
                        nc.tensor.matmul(
                            pv[:, j],
                            lhsT=v_sb[:, j, i],
                            rhs=wv(i),
                            start=(i == 0),
                            stop=(i == MB - 1),
                        )
                nc.vector.tensor_copy(vaug[:, :, 0:D_K], pv)

            SB = {}
            Es = {}

            def scores_g(c, g):
                """Score MMs + fused mask-eviction (DVE) for group g (pairs 2g, 2g+1)."""
                csl = slice(c * 512, (c + 1) * 512)
                sb = epool.tile([P, 4, 512], bf16, tag=f"S{g}", name=f"S{g}_{c}")
                SB[(g, c)] = sb
                for ph in range(2):
                    p = 2 * g + ph
                    sc = ps_s.tile([P, 2, 512], f32, tag="sc", name=f"sc{p}_{c}")
                    for h in range(2):
                        j = 2 * p + h
                        nc.tensor.matmul(
                            sc[:, h],
                            lhsT=kT[:, j * P : (j + 1) * P],
                            rhs=qT[:, csl],
                            start=True,
                            stop=True,
                        )
                    nc.vector.tensor_tensor(
                        out=sb[:, 2 * ph : 2 * ph + 2],
                        in0=sc,
                        in1=m_sb[:, c, 2 * p : 2 * p + 2],
                        op=Alu.mult,
                    )

            def exp_g(c, g):
                E = epool.tile([P, 4, 512], bf16, tag=f"E{g}", name=f"E{g}_{c}")
                nc.scalar.activation(
                    out=E,
                    in_=SB[(g, c)],
                    func=mybir.ActivationFunctionType.Exp,
                    scale=float(D_MODEL) ** -0.5,
                )
                Es[(g, c)] = E

            oT_sb = fpool.tile([D_K + 1, NQC, 512], bf16, tag="oT")

            def pv_chunk(c):
                po = ps_o.tile([D_K + 1, 512], f32, tag="o", name=f"o{c}")
                for j in range(JT):
                    nc.tensor.matmul(
                        po,
                        lhsT=vaug[:, j],
                        rhs=Es[(j // 4, c)][:, j % 4],
                        start=(j == 0),
                        stop=(j == JT - 1),
                    )
                nc.scalar.copy(oT_sb[:, c], po)
                nc.sync.dma_start(out=out_d[:, c], in_=oT_sb[:, c])

            # ---- emission in intended engine order ----
            kproj(0)
            qproj(0)
            scores_g(0, 0)
            exp_g(0, 0)
            kproj(1)
            scores_g(0, 1)
            exp_g(0, 1)
            qproj(1)
            scores_g(1, 0)
            exp_g(1, 0)
            scores_g(1, 1)
            exp_g(1, 1)
            qproj(2)
            scores_g(2, 0)
            exp_g(2, 0)
            scores_g(2, 1)
            exp_g(2, 1)
            vproj()
            pv_chunk(0)
            pv_chunk(1)
            qproj(3)
            scores_g(3, 0)
            exp_g(3, 0)
            scores_g(3, 1)
            exp_g(3, 1)
            pv_chunk(2)
            pv_chunk(3)

    nc.finalize()
    return nc


def _get_nc():
    global _cached_nc
    if _cached_nc is None:
        _cached_nc = _build_nc()
    return _cached_nc


def _pack_chunks(x_t, nchunks, dtype):
    s = x_t.shape[1]
    assert s == nchunks * 512
    return np.ascontiguousarray(
        x_t.astype(dtype).reshape(MB, P, nchunks, 512).transpose(1, 2, 0, 3)
    )


def _shard_inputs(query, key, value, mask, w_q, w_k, w_v):
    wq_dev = np.ascontiguousarray(
        w_q.T.astype(_FP8).reshape(MB, P, D_K).transpose(1, 0, 2)
    )
    wkv_dev = np.ascontiguousarray(
        np.concatenate(
            [
                w.T.astype(_BF16).reshape(MB, P, D_K).transpose(1, 0, 2)
                for w in (w_k, w_v)
            ],
            axis=2,
        )
    )
    in_maps = []
    for c in range(N_CORES):
        b, h = divmod(c, 2)
        s0 = h * SKV
        q_t = query[b].T
        k_t = key[b, s0 : s0 + SKV, :].T
        v_t = value[b, s0 : s0 + SKV, :].T.astype(_BF16)
        m_inv = (~mask[b, :, s0 : s0 + SKV]).astype(np.uint8).T
        m_dev = np.ascontiguousarray(
            m_inv.reshape(JT, P, NQC, 512).transpose(1, 2, 0, 3)
        )
        in_maps.append(
            {
                "w_q": wq_dev,
                "w_kv": wkv_dev,
                "q_t": _pack_chunks(q_t, NQC, _FP8),
                "k_t": _pack_chunks(k_t, NKC, _BF16),
                "v_t": np.ascontiguousarray(
                    v_t.reshape(MB, P, JT, P).transpose(1, 2, 0, 3)
                ),
                "m_t": m_dev,
            }
        )
    return in_maps


def run(inputs, trace=False):
    from concourse.bass_utils import run_bass_kernel_spmd

    nc = _get_nc()
    in_maps = _shard_inputs(**inputs)
    res = run_bass_kernel_spmd(
        nc, in_maps, core_ids=list(range(N_CORES)), trace=trace
    )

    out = np.empty((B, S, D_K), np.float32)
    for b in range(B):
        numer = np.zeros((D_K, S), np.float32)
        denom = np.zeros((S,), np.float32)
        for h in range(2):
            c = 2 * b + h
            o = np.asarray(res.results[c]["out"], np.float32).reshape(D_K + 1, S)
            numer += o[0:D_K]
            denom += o[D_K]
        out[b] = (numer / denom[None, :]).T
    return out, res


def kernel(**inputs):
    out, _ = run(inputs, trace=False)
    return out


# revision 3
# speedup vs baseline: 1.1477x; 1.0274x over previous
"""AttentionHead kernel v5 for 8 Trainium2 NeuronCores (SPMD data-parallel).

Sharding (kv-shard): 8 cores = 4 batches x 2 KV-halves. Each core:
full query [2048, 1024] in FP8(e4m3), half key/value [1024, 1024] bf16,
inverted mask u8 [skv, sq]. Per-core DMA ~8.3 MiB.

Per-core pipeline (chunk c = 512 query columns, group g = 4 j-tiles):
  - kT/qT projections (M=64); qproj runs in fp8 (query + w_q quantized
    e4m3; only the q side is fp8 -- error ~1.3% rel, within the 2e-2
    budget); PSUM->SBUF evictions on ACT.
  - vproj: all 8 j into ONE PSUM bank, single DVE cast into vaug
    [128, 8, 65] (ones col).
  - scores per (j, c): [128, 512] transposed, pair PSUM tiles
    [128, 2, 512]; mask fused into the PSUM->SBUF eviction: ONE DVE
    tensor_tensor per pair: sb = scores * m' (u8). Masked -> exp(0)=1.
  - ACT exp per (g, c) on [128, 4, 512] SBUF bf16 (FD=2048).
  - PV: oT_c[65, 512] += vaug_j^T @ E; row 64 = denominator; oT
    eviction on DVE.
  - host: numer = oT[0:64], denom = oT[64], combine halves, divide.
"""

import numpy as np
import ml_dtypes

B = 4
S = 2048
D_MODEL = 1024
D_K = 64
N_CORES = 8

P = 128
SQ = S
SKV = S // 2
MB = D_MODEL // P
JT = SKV // P
NQC = SQ // 512
NKC = SKV // 512
NG = 2  # j-tile groups per chunk (4 j each)

_BF16 = ml_dtypes.bfloat16
_FP8 = ml_dtypes.float8_e4m3

_cached_nc = None


def _build_nc():
    import concourse.mybir as mybir
    import concourse.tile as tile
    from concourse import bacc

    bf16 = mybir.dt.bfloat16
    fp8 = mybir.dt.float8e4
    f32 = mybir.dt.float32
    u8 = mybir.dt.uint8
    Alu = mybir.AluOpType

    nc = bacc.Bacc(None, target_bir_lowering=False)

    wq_d = nc.dram_tensor("w_q", [P, MB, D_K], fp8, kind="ExternalInput")
    wkv_d = nc.dram_tensor("w_kv", [P, MB, 2 * D_K], bf16, kind="ExternalInput")
    q_d = nc.dram_tensor("q_t", [P, NQC, MB, 512], fp8, kind="ExternalInput")
    k_d = nc.dram_tensor("k_t", [P, NKC, MB, 512], bf16, kind="ExternalInput")
    v_d = nc.dram_tensor("v_t", [P, JT, MB, P], bf16, kind="ExternalInput")
    m_d = nc.dram_tensor("m_t", [P, NQC, JT, 512], u8, kind="ExternalInput")
    out_d = nc.dram_tensor("out", [D_K + 1, NQC, 512], bf16, kind="ExternalOutput")

    with tile.TileContext(nc) as tc:
        with (
            tc.tile_pool(name="const", bufs=1) as cpool,
            tc.tile_pool(name="inp", bufs=1) as ipool,
            tc.tile_pool(name="proj", bufs=1) as jpool,
            tc.tile_pool(name="ee", bufs=2) as epool,
            tc.tile_pool(name="ee3", bufs=3) as epool3,
            tc.tile_pool(name="fin", bufs=1) as fpool,
            tc.tile_pool(name="ps_pqk", bufs=2, space="PSUM") as ps_pqk,
            tc.tile_pool(name="ps_s", bufs=2, space="PSUM") as ps_s,
            tc.tile_pool(name="ps_o", bufs=1, space="PSUM") as ps_o,
            tc.tile_pool(name="ps_v", bufs=1, space="PSUM") as ps_v,
        ):
            k_sb = ipool.tile([P, NKC, MB, 512], bf16, tag="k")
            q_sb = ipool.tile([P, NQC, MB, 512], fp8, tag="q")
            m_sb = ipool.tile([P, NQC, JT, 512], u8, tag="m")
            v_sb = ipool.tile([P, JT, MB, P], bf16, tag="v")
            wq_sb = cpool.tile([P, MB, D_K], fp8, tag="wq")
            wkv_sb = cpool.tile([P, MB, 2 * D_K], bf16, tag="wkv")

            def dma_k(kc):
                nc.sync.dma_start(out=k_sb[:, kc], in_=k_d[:, kc])

            def dma_q(c):
                nc.sync.dma_start(out=q_sb[:, c], in_=q_d[:, c])

            def dma_m(c):
                nc.sync.dma_start(out=m_sb[:, c], in_=m_d[:, c])

            def dma_v(g):
                nc.sync.dma_start(
                    out=v_sb[:, g * 4 : (g + 1) * 4], in_=v_d[:, g * 4 : (g + 1) * 4]
                )

            dma_k(0)
            nc.sync.dma_start(out=wkv_sb, in_=wkv_d[:])
            nc.sync.dma_start(out=wq_sb, in_=wq_d[:])
            dma_q(0)
            dma_m(0)
            dma_k(1)
            dma_q(1)
            dma_m(1)
            dma_q(2)
            dma_m(2)
            dma_v(0)
            dma_v(1)
            dma_q(3)
            dma_m(3)

            def wq(i):
                return wq_sb[:, i]

            def wk(i):
                return wkv_sb[:, i, 0:D_K]

            def wv(i):
                return wkv_sb[:, i, D_K : 2 * D_K]

            warm = cpool.tile([P, 512], bf16, tag="warm")
            nc.vector.memset(warm, 0.25)
            vaug = jpool.tile([P, JT, D_K + 1], bf16, tag="vaug")
            nc.vector.memset(vaug[:, :, D_K : D_K + 1], 1.0)

            warm_ps = ps_pqk.tile([D_K, 512], f32, tag="pqk", name="warm")
            for wi in range(6):
                nc.tensor.matmul(
                    warm_ps,
                    lhsT=warm[:, 0:D_K],
                    rhs=warm,
                    start=(wi == 0),
                    stop=(wi == 5),
                )

            qT = jpool.tile([D_K, SQ], bf16, tag="qT")
            kT = jpool.tile([D_K, SKV], bf16, tag="kT")

            def kproj(kc):
                pp = ps_pqk.tile([D_K, 512], f32, tag="pqk", name=f"pk{kc}")
                for i in range(MB):
                    nc.tensor.matmul(
                        pp, lhsT=wk(i), rhs=k_sb[:, kc, i], start=(i == 0), stop=(i == MB - 1)
                    )
                # front evictions ride the still-idle DVE, off ACT's stream
                nc.vector.tensor_copy(kT[:, kc * 512 : (kc + 1) * 512], pp)

            def qproj(c):
                pp = ps_pqk.tile([D_K, 512], f32, tag="pqk", name=f"pq{c}")
                for i in range(MB):
                    nc.tensor.matmul(
                        pp, lhsT=wq(i), rhs=q_sb[:, c, i], start=(i == 0), stop=(i == MB - 1)
                    )
                if c == 0:
                    nc.vector.tensor_copy(qT[:, 0:512], pp)
                else:
                    nc.scalar.copy(qT[:, c * 512 : (c + 1) * 512], pp)

            def vproj():
                pv = ps_v.tile([P, JT, D_K], f32, tag="pv", name="pv")
                for j in range(JT):
                    for i in range(MB):
                        nc.tensor.matmul(
                            pv[:, j],
                            lhsT=v_sb[:, j, i],
                            rhs=wv(i),
                            start=(i == 0),
                            stop=(i == MB - 1),
                        )
                nc.vector.tensor_copy(vaug[:, :, 0:D_K], pv)

            SB = {}
            Es = {}

            def scores_g(c, g):
                """Score MMs + fused mask-eviction (DVE) for group g (pairs 2g, 2g+1)."""
                csl = slice(c * 512, (c + 1) * 512)
                for ph in range(2):
                    p = 2 * g + ph
                    sc = ps_s.tile([P, 2, 512], f32, tag="sc", name=f"sc{p}_{c}")
                    for h in range(2):
                        j = 2 * p + h
                        nc.tensor.matmul(
                            sc[:, h],
                            lhsT=kT[:, j * P : (j + 1) * P],
                            rhs=qT[:, csl],
                            start=True,
                            stop=True,
                        )
                    sb = epool.tile([P, 2, 512], bf16, tag=f"S{p}", name=f"S{p}_{c}")
                    SB[(p, c)] = sb
                    nc.vector.tensor_tensor(
                        out=sb,
                        in0=sc,
                        in1=m_sb[:, c, 2 * p : 2 * p + 2],
                        op=Alu.mult,
                    )

            def exp_g(c, g):
                for ph in range(2):
                    p = 2 * g + ph
                    E = epool3.tile([P, 2, 512], bf16, tag=f"E{p}", name=f"E{p}_{c}")
                    nc.scalar.activation(
                        out=E,
                        in_=SB[(p, c)],
                        func=mybir.ActivationFunctionType.Exp,
                        scale=float(D_MODEL) ** -0.5,
                    )
                    Es[(p, c)] = E

            oT_sb = fpool.tile([D_K + 1, NQC, 512], bf16, tag="oT")

            def pv_chunk(c):
                po = ps_o.tile([D_K + 1, 512], f32, tag="o", name=f"o{c}")
                for j in range(JT):
                    nc.tensor.matmul(
                        po,
                        lhsT=vaug[:, j],
                        rhs=Es[(j // 2, c)][:, j % 2],
                        start=(j == 0),
                        stop=(j == JT - 1),
                    )
                nc.scalar.copy(oT_sb[:, c], po)
                nc.sync.dma_start(out=out_d[:, c], in_=oT_sb[:, c])

            # ---- emission in intended engine order ----
            kproj(0)
            qproj(0)
            scores_g(0, 0)
            exp_g(0, 0)
            kproj(1)
            scores_g(0, 1)
            exp_g(0, 1)
            qproj(1)
            scores_g(1, 0)
            exp_g(1, 0)
            scores_g(1, 1)
            exp_g(1, 1)
            qproj(2)
            scores_g(2, 0)
            exp_g(2, 0)
            scores_g(2, 1)
            exp_g(2, 1)
            vproj()
            qproj(3)
            scores_g(3, 0)
            exp_g(3, 0)
            scores_g(3, 1)
            exp_g(3, 1)
            pv_chunk(0)
            pv_chunk(1)
            pv_chunk(2)
            pv_chunk(3)

    nc.finalize()
    return nc


def _get_nc():
    global _cached_nc
    if _cached_nc is None:
        _cached_nc = _build_nc()
    return _cached_nc


def _pack_chunks(x_t, nchunks, dtype):
    s = x_t.shape[1]
    assert s == nchunks * 512
    return np.ascontiguousarray(
        x_t.astype(dtype).reshape(MB, P, nchunks, 512).transpose(1, 2, 0, 3)
    )


def _shard_inputs(query, key, value, mask, w_q, w_k, w_v):
    wq_dev = np.ascontiguousarray(
        w_q.T.astype(_FP8).reshape(MB, P, D_K).transpose(1, 0, 2)
    )
    wkv_dev = np.ascontiguousarray(
        np.concatenate(
            [
                w.T.astype(_BF16).reshape(MB, P, D_K).transpose(1, 0, 2)
                for w in (w_k, w_v)
            ],
            axis=2,
        )
    )
    in_maps = []
    for c in range(N_CORES):
        b, h = divmod(c, 2)
        s0 = h * SKV
        q_t = query[b].T
        k_t = key[b, s0 : s0 + SKV, :].T
        v_t = value[b, s0 : s0 + SKV, :].T.astype(_BF16)
        m_inv = (~mask[b, :, s0 : s0 + SKV]).astype(np.uint8).T
        m_dev = np.ascontiguousarray(
            m_inv.reshape(JT, P, NQC, 512).transpose(1, 2, 0, 3)
        )
        in_maps.append(
            {
                "w_q": wq_dev,
                "w_kv": wkv_dev,
                "q_t": _pack_chunks(q_t, NQC, _FP8),
                "k_t": _pack_chunks(k_t, NKC, _BF16),
                "v_t": np.ascontiguousarray(
                    v_t.reshape(MB, P, JT, P).transpose(1, 2, 0, 3)
                ),
                "m_t": m_dev,
            }
        )
    return in_maps


def run(inputs, trace=False):
    from concourse.bass_utils import run_bass_kernel_spmd

    nc = _get_nc()
    in_maps = _shard_inputs(**inputs)
    res = run_bass_kernel_spmd(
        nc, in_maps, core_ids=list(range(N_CORES)), trace=trace
    )

    out = np.empty((B, S, D_K), np.float32)
    for b in range(B):
        numer = np.zeros((D_K, S), np.float32)
        denom = np.zeros((S,), np.float32)
        for h in range(2):
            c = 2 * b + h
            o = np.asarray(res.results[c]["out"], np.float32).reshape(D_K + 1, S)
            numer += o[0:D_K]
            denom += o[D_K]
        out[b] = (numer / denom[None, :]).T
    return out, res


def kernel(**inputs):
    out, _ = run(inputs, trace=False)
    return out


# revision 4
# speedup vs baseline: 1.1670x; 1.0168x over previous
"""AttentionHead kernel v5 for 8 Trainium2 NeuronCores (SPMD data-parallel).

Sharding (kv-shard): 8 cores = 4 batches x 2 KV-halves. Each core:
full query [2048, 1024] in FP8(e4m3), half key/value [1024, 1024] bf16,
inverted mask u8 [skv, sq]. Per-core DMA ~8.3 MiB.

Per-core pipeline (chunk c = 512 query columns, group g = 4 j-tiles):
  - kT/qT projections (M=64); qproj runs in fp8 (query + w_q quantized
    e4m3; only the q side is fp8 -- error ~1.3% rel, within the 2e-2
    budget); PSUM->SBUF evictions on ACT.
  - vproj: all 8 j into ONE PSUM bank, single DVE cast into vaug
    [128, 8, 65] (ones col).
  - scores per (j, c): [128, 512] transposed, pair PSUM tiles
    [128, 2, 512]; mask fused into the PSUM->SBUF eviction: ONE DVE
    tensor_tensor per pair: sb = scores * m' (u8). Masked -> exp(0)=1.
  - ACT exp per (g, c) on [128, 4, 512] SBUF bf16 (FD=2048).
  - PV: oT_c[65, 512] += vaug_j^T @ E; row 64 = denominator; oT
    eviction on DVE.
  - host: numer = oT[0:64], denom = oT[64], combine halves, divide.
"""

import numpy as np
import ml_dtypes

B = 4
S = 2048
D_MODEL = 1024
D_K = 64
N_CORES = 8

P = 128
SQ = S
SKV = S // 2
MB = D_MODEL // P
JT = SKV // P
NQC = SQ // 512
NKC = SKV // 512
NG = 2  # j-tile groups per chunk (4 j each)

_BF16 = ml_dtypes.bfloat16
_FP8 = ml_dtypes.float8_e4m3

_cached_nc = None


def _build_nc():
    import concourse.mybir as mybir
    import concourse.tile as tile
    from concourse import bacc

    bf16 = mybir.dt.bfloat16
    fp8 = mybir.dt.float8e4
    f32 = mybir.dt.float32
    u8 = mybir.dt.uint8
    Alu = mybir.AluOpType

    nc = bacc.Bacc(None, target_bir_lowering=False)

    wq_d = nc.dram_tensor("w_q", [P, MB, D_K], fp8, kind="ExternalInput")
    wkv_d = nc.dram_tensor("w_kv", [P, MB, 2 * D_K], bf16, kind="ExternalInput")
    q_d = nc.dram_tensor("q_t", [P, NQC, MB, 512], fp8, kind="ExternalInput")
    k_d = nc.dram_tensor("k_t", [P, NKC, MB, 512], bf16, kind="ExternalInput")
    v_d = nc.dram_tensor("v_t", [P, JT, MB, P], bf16, kind="ExternalInput")
    m_d = nc.dram_tensor("m_t", [P, NQC, JT, 512], u8, kind="ExternalInput")
    out_d = nc.dram_tensor("out", [D_K + 1, NQC, 512], bf16, kind="ExternalOutput")

    with tile.TileContext(nc) as tc:
        with (
            tc.tile_pool(name="const", bufs=1) as cpool,
            tc.tile_pool(name="inp", bufs=1) as ipool,
            tc.tile_pool(name="proj", bufs=1) as jpool,
            tc.tile_pool(name="ee", bufs=2) as epool,
            tc.tile_pool(name="ee3", bufs=3) as epool3,
            tc.tile_pool(name="fin", bufs=1) as fpool,
            tc.tile_pool(name="ps_pqk", bufs=2, space="PSUM") as ps_pqk,
            tc.tile_pool(name="ps_s", bufs=2, space="PSUM") as ps_s,
            tc.tile_pool(name="ps_o", bufs=2, space="PSUM") as ps_o,
        ):
            k_sb = ipool.tile([P, NKC, MB, 512], bf16, tag="k")
            q_sb = ipool.tile([P, NQC, MB, 512], fp8, tag="q")
            m_sb = ipool.tile([P, NQC, JT, 512], u8, tag="m")
            v_sb = ipool.tile([P, JT, MB, P], bf16, tag="v")
            wq_sb = cpool.tile([P, MB, D_K], fp8, tag="wq")
            wkv_sb = cpool.tile([P, MB, 2 * D_K], bf16, tag="wkv")

            def dma_k(kc):
                nc.sync.dma_start(out=k_sb[:, kc], in_=k_d[:, kc])

            def dma_q(c):
                nc.sync.dma_start(out=q_sb[:, c], in_=q_d[:, c])

            def dma_m(c):
                nc.sync.dma_start(out=m_sb[:, c], in_=m_d[:, c])

            def dma_v(g):
                nc.sync.dma_start(
                    out=v_sb[:, g * 4 : (g + 1) * 4], in_=v_d[:, g * 4 : (g + 1) * 4]
                )

            dma_k(0)
            nc.sync.dma_start(out=wkv_sb, in_=wkv_d[:])
            nc.sync.dma_start(out=wq_sb, in_=wq_d[:])
            dma_q(0)
            dma_m(0)
            dma_k(1)
            dma_q(1)
            dma_m(1)
            dma_q(2)
            dma_m(2)
            dma_v(0)
            dma_v(1)
            dma_q(3)
            dma_m(3)

            def wq(i):
                return wq_sb[:, i]

            def wk(i):
                return wkv_sb[:, i, 0:D_K]

            def wv(i):
                return wkv_sb[:, i, D_K : 2 * D_K]

            warm = cpool.tile([P, 512], bf16, tag="warm")
            nc.vector.memset(warm, 0.25)
            vaug = jpool.tile([P, JT, D_K + 1], bf16, tag="vaug")
            nc.vector.memset(vaug[:, :, D_K : D_K + 1], 1.0)

            warm_ps = ps_pqk.tile([P, 512], f32, tag="pqk", name="warm")
            for wi in range(6):
                nc.tensor.matmul(
                    warm_ps[0:D_K],
                    lhsT=warm[:, 0:D_K],
                    rhs=warm,
                    start=(wi == 0),
                    stop=(wi == 5),
                )

            qT = jpool.tile([D_K, SQ], bf16, tag="qT")
            kT = jpool.tile([D_K, SKV], bf16, tag="kT")

            def kproj(kc):
                pp = ps_pqk.tile([P, 512], f32, tag="pqk", name=f"pk{kc}")
                for i in range(MB):
                    nc.tensor.matmul(
                        pp[0:D_K],
                        lhsT=wk(i),
                        rhs=k_sb[:, kc, i],
                        start=(i == 0),
                        stop=(i == MB - 1),
                    )
                # front evictions ride the still-idle DVE, off ACT's stream
                nc.vector.tensor_copy(kT[:, kc * 512 : (kc + 1) * 512], pp[0:D_K])

            def qproj(c):
                pp = ps_pqk.tile([P, 512], f32, tag="pqk", name=f"pq{c}")
                for i in range(MB):
                    nc.tensor.matmul(
                        pp[0:D_K],
                        lhsT=wq(i),
                        rhs=q_sb[:, c, i],
                        start=(i == 0),
                        stop=(i == MB - 1),
                    )
                if c == 0:
                    nc.vector.tensor_copy(qT[:, 0:512], pp[0:D_K])
                else:
                    nc.scalar.copy(qT[:, c * 512 : (c + 1) * 512], pp[0:D_K])

            def vproj():
                pv = ps_pqk.tile([P, 512], f32, tag="pqk", name="pv")
                pvj = pv.rearrange("p (j k) -> p j k", j=JT)
                for j in range(JT):
                    for i in range(MB):
                        nc.tensor.matmul(
                            pvj[:, j],
                            lhsT=v_sb[:, j, i],
                            rhs=wv(i),
                            start=(i == 0),
                            stop=(i == MB - 1),
                        )
                nc.vector.tensor_copy(vaug[:, :, 0:D_K], pvj)

            SB = {}
            Es = {}

            def scores_g(c, g):
                """Score MMs + fused mask-eviction (DVE) for group g (pairs 2g, 2g+1)."""
                csl = slice(c * 512, (c + 1) * 512)
                for ph in range(2):
                    p = 2 * g + ph
                    sc = ps_s.tile([P, 2, 512], f32, tag="sc", name=f"sc{p}_{c}")
                    for h in range(2):
                        j = 2 * p + h
                        nc.tensor.matmul(
                            sc[:, h],
                            lhsT=kT[:, j * P : (j + 1) * P],
                            rhs=qT[:, csl],
                            start=True,
                            stop=True,
                        )
                    sb = epool.tile([P, 2, 512], bf16, tag=f"S{p}", name=f"S{p}_{c}")
                    SB[(p, c)] = sb
                    nc.vector.tensor_tensor(
                        out=sb,
                        in0=sc,
                        in1=m_sb[:, c, 2 * p : 2 * p + 2],
                        op=Alu.mult,
                    )

            def exp_g(c, g):
                for ph in range(2):
                    p = 2 * g + ph
                    E = epool3.tile([P, 2, 512], bf16, tag=f"E{p}", name=f"E{p}_{c}")
                    nc.scalar.activation(
                        out=E,
                        in_=SB[(p, c)],
                        func=mybir.ActivationFunctionType.Exp,
                        scale=float(D_MODEL) ** -0.5,
                    )
                    Es[(p, c)] = E

            oT_sb = fpool.tile([D_K + 1, NQC, 512], bf16, tag="oT")

            def pv_chunk(c):
                po = ps_o.tile([D_K + 1, 512], f32, tag="o", name=f"o{c}")
                for j in range(JT):
                    nc.tensor.matmul(
                        po,
                        lhsT=vaug[:, j],
                        rhs=Es[(j // 2, c)][:, j % 2],
                        start=(j == 0),
                        stop=(j == JT - 1),
                    )
                nc.scalar.copy(oT_sb[:, c], po)
                nc.sync.dma_start(out=out_d[:, c], in_=oT_sb[:, c])

            # ---- emission in intended engine order ----
            kproj(0)
            qproj(0)
            scores_g(0, 0)
            exp_g(0, 0)
            kproj(1)
            scores_g(0, 1)
            exp_g(0, 1)
            qproj(1)
            scores_g(1, 0)
            exp_g(1, 0)
            scores_g(1, 1)
            exp_g(1, 1)
            qproj(2)
            scores_g(2, 0)
            exp_g(2, 0)
            scores_g(2, 1)
            exp_g(2, 1)
            vproj()
            qproj(3)
            scores_g(3, 0)
            exp_g(3, 0)
            scores_g(3, 1)
            exp_g(3, 1)
            pv_chunk(0)
            pv_chunk(1)
            pv_chunk(2)
            pv_chunk(3)

    nc.finalize()
    return nc


def _get_nc():
    global _cached_nc
    if _cached_nc is None:
        _cached_nc = _build_nc()
    return _cached_nc


def _pack_chunks(x_t, nchunks, dtype):
    s = x_t.shape[1]
    assert s == nchunks * 512
    return np.ascontiguousarray(
        x_t.astype(dtype).reshape(MB, P, nchunks, 512).transpose(1, 2, 0, 3)
    )


def _shard_inputs(query, key, value, mask, w_q, w_k, w_v):
    wq_dev = np.ascontiguousarray(
        w_q.T.astype(_FP8).reshape(MB, P, D_K).transpose(1, 0, 2)
    )
    wkv_dev = np.ascontiguousarray(
        np.concatenate(
            [
                w.T.astype(_BF16).reshape(MB, P, D_K).transpose(1, 0, 2)
                for w in (w_k, w_v)
            ],
            axis=2,
        )
    )
    in_maps = []
    for c in range(N_CORES):
        b, h = divmod(c, 2)
        s0 = h * SKV
        q_t = query[b].T
        k_t = key[b, s0 : s0 + SKV, :].T
        v_t = value[b, s0 : s0 + SKV, :].T.astype(_BF16)
        m_inv = (~mask[b, :, s0 : s0 + SKV]).astype(np.uint8).T
        m_dev = np.ascontiguousarray(
            m_inv.reshape(JT, P, NQC, 512).transpose(1, 2, 0, 3)
        )
        in_maps.append(
            {
                "w_q": wq_dev,
                "w_kv": wkv_dev,
                "q_t": _pack_chunks(q_t, NQC, _FP8),
                "k_t": _pack_chunks(k_t, NKC, _BF16),
                "v_t": np.ascontiguousarray(
                    v_t.reshape(MB, P, JT, P).transpose(1, 2, 0, 3)
                ),
                "m_t": m_dev,
            }
        )
    return in_maps


def run(inputs, trace=False):
    from concourse.bass_utils import run_bass_kernel_spmd

    nc = _get_nc()
    in_maps = _shard_inputs(**inputs)
    res = run_bass_kernel_spmd(
        nc, in_maps, core_ids=list(range(N_CORES)), trace=trace
    )

    out = np.empty((B, S, D_K), np.float32)
    for b in range(B):
        numer = np.zeros((D_K, S), np.float32)
        denom = np.zeros((S,), np.float32)
        for h in range(2):
            c = 2 * b + h
            o = np.asarray(res.results[c]["out"], np.float32).reshape(D_K + 1, S)
            numer += o[0:D_K]
            denom += o[D_K]
        out[b] = (numer / denom[None, :]).T
    return out, res


def kernel(**inputs):
    out, _ = run(inputs, trace=False)
    return out
